# revision 41
# baseline (speedup 1.0000x reference)
"""KV-cached multi-head attention on 8 Trainium2 NeuronCores.

Sharding: 4-way batch (data parallel) x 2-way heads (tensor parallel).
Core c handles batch b = c//2 and head-half h2 = c%2 (8 of 16 heads).
Each core: Q/K/V projections (column-sharded), 8-head causal attention
against the concatenated KV cache, and a row-sharded out-projection
partial. The two partials per batch are summed on the host (+ bo).

The Q/K/V projections and the out-projection run as 3-term hi/lo fp8
(e4m3) DoubleRow matmuls: each operand X is split X = Xh + Xl with both
parts fp8, and X@W is evaluated as Xh@Wh + Xl@Wh + Xh@Wl (the lo*lo
term is ~2^-9 relative and dropped). DoubleRow packs two 128-deep
contraction tiles per instruction at 0.5 cycles/row, so the three
terms cost 0.75x the bf16 schedule at bf16-level accuracy. x and W
splits are host-prepped, pre-tiled to the exact SBUF layouts (>=2KB
contiguous descriptors; sub-512B DMA elements pay a 2x latency
multiplier), with weights prescaled into e4m3's normal range (the
drain rescales). The attention output's split is produced on the fly:
DVE writes o*rec*16 to bf16, GPSIMD casts the fp8 hi and subtracts
for the lo.

Attention itself (scores, exp, softmax reduction, PV) stays bf16: with
a 128-deep score contraction DoubleRow needs all four hi/lo cross
terms, which is cycle-neutral, and single-fp8 operands fail the
accuracy budget on peaked softmax rows. The schedule keeps:
  - One shared 8-bank PSUM pool, retagged per phase. Projections use
    eight one-bank [P,512] accumulators; a DoubleRow matmul writes 256
    cols, and the two halves of a bank share ONE accumulation group:
    the first instruction's start=True zeroes the whole 2KB zero
    region and the second half accumulates with start=False (half-bank
    groups with their own start are WRONG - start zeroing is
    region-wide). Attention rotates [P,1024] score PAIRS on d0/d1,
    O^T on o0/o1; the deferred Q chunk-1 projection and the
    out-projection rotate x0/x1.
  - Paired exps: two score tiles share one 2-bank PSUM tile and one
    [P,1024] ACT exp lands in two adjacent ptb slots.
  - Softmax denominator off the PE: bf16 pairwise TT-add tree on DVE
    (2x mode) + causally-restricted diagonal adds, GPSIMD
    partition_all_reduce, DVE reciprocal deferred one chunk.
  - K/Q biases ride the PSUM drain (ACT/DVE split); V-bias folded to
    the host (cv - bv, bo + bv @ Wo.T) since softmax weights sum to 1.
  - Q chunk-1 and the Wo prefetch are deferred into the attention-c0
    window; attention PVs flow through a cross-chunk software pipeline.
"""

import sys

sys.path.insert(0, "/opt/trn_rl_repo")

import numpy as np
import ml_dtypes

import concourse.bass as bass  # noqa: F401  (registers AP types)
import concourse.mybir as mybir
import concourse.tile as tile
from concourse import bacc
from concourse import bass_isa
from concourse.bass_utils import run_bass_kernel_spmd

F32 = mybir.dt.float32
BF16 = mybir.dt.bfloat16
FP8 = mybir.dt.float8e4
BF = ml_dtypes.bfloat16
E4 = ml_dtypes.float8_e4m3

D = 2048          # model dim
SQ = 1024         # new tokens per batch
SC = 1024         # cached tokens
SKV = SC + SQ     # total keys
HD = 128          # head dim
HLOC = 8          # heads per core
DH = HLOC * HD    # per-core projected dim (1024)
NCORES = 8
P = 128

SW = 64.0         # host prescale on Wk/Wv/Wo (into e4m3 normal range)
SWQ = 512.0       # host prescale on Wq (which also carries 1/sqrt(hd))
SA = 16.0         # on-chip prescale on the attention output

EXP = mybir.ActivationFunctionType.Exp
IDENT = mybir.ActivationFunctionType.Identity
DR = mybir.MatmulPerfMode.DoubleRow
MUL = mybir.AluOpType.mult
ADD = mybir.AluOpType.add


def _emit(tc, nc, prm):
    with (
        tc.tile_pool(name="res", bufs=1) as res,
        tc.tile_pool(name="wres", bufs=1) as wres,
        tc.tile_pool(name="xs", bufs=2) as xs,
        tc.tile_pool(name="ptp", bufs=2) as ptp,
        tc.tile_pool(name="trp", bufs=2) as trp,
        tc.tile_pool(name="accp", bufs=2) as accp,
        tc.tile_pool(name="dnp", bufs=2) as dnp,
        tc.tile_pool(name="abf", bufs=2) as abf,
        tc.tile_pool(name="outs", bufs=4) as outs,
        tc.tile_pool(name="ps8", bufs=1, space="PSUM") as ps8,
    ):
        qt = [res.tile([P, SQ], BF16, name=f"qt{h}", tag=f"qt{h}") for h in range(HLOC)]
        kt = [res.tile([P, SKV], BF16, name=f"kt{h}", tag=f"kt{h}") for h in range(HLOC)]
        vv = [res.tile([P, DH], BF16, name=f"vv{t}", tag=f"vv{t}") for t in range(16)]
        # attention output, fp8 hi/lo, head pairs in the middle dim for
        # the out-projection's DoubleRow contraction pairs
        ath = [res.tile([P, 2, SQ], FP8, name=f"ath{j}", tag=f"ath{j}") for j in range(4)]
        atl = [res.tile([P, 2, SQ], FP8, name=f"atl{j}", tag=f"atl{j}") for j in range(4)]
        tri = res.tile([P, P], BF16, name="tri", tag="tri")
        bq2 = res.tile([P, 8], F32, name="bq2", tag="bq2")
        bk2 = res.tile([P, 8], F32, name="bk2", tag="bk2")

        # prologue on the Pool (SWDGE) queue: constants first, then KV-cache
        # loads that trickle in during the projections.
        warm = ps8.tile([P, 512], F32, name="warm", tag="x1")
        nc.tensor.matmul(warm[0:1, 0:1], tri[:, 0:1], tri[:, 0:1], start=True, stop=True)
        nc.gpsimd.dma_start(tri[:], prm["tri"][:])
        nc.gpsimd.dma_start(bq2[:], prm["bq2"][:])
        nc.gpsimd.dma_start(bk2[:], prm["bk2"][:])
        # stream tile: 16 contraction tiles x [hi 256 | lo 256], loaded
        # from a host-pre-tiled [4, P, 16, 512] param in two half DMAs
        def stream_tile(name):
            return xs.tile([P, 16, 512], FP8, name=name, tag="xs")

        def load_stream(t_, par, c4, q=None):
            q = q or nc.scalar
            q.dma_start(t_[:, 0:8, :], prm[par][c4, :, 0:8, :])
            q.dma_start(t_[:, 8:16, :], prm[par][c4, :, 8:16, :])

        # 8 accumulators, one full 2KB PSUM bank each. A DoubleRow matmul
        # writes 256 cols; the two 256-col halves of a bank share ONE
        # accumulation group: the very first instruction's start=True
        # zeroes the whole 2KB zero region, the second half accumulates
        # onto the pre-zeroed region with start=False.
        def proj_ps(pref):
            d0 = ps8.tile([P, 1024], F32, name=f"{pref}d0", tag="d0")
            d1 = ps8.tile([P, 1024], F32, name=f"{pref}d1", tag="d1")
            bts = [ps8.tile([P, 512], F32, name=f"{pref}b{i}", tag=t)
                   for i, t in enumerate(("o0", "o1", "x0", "x1"))]
            return [d0[:, 0:512], d0[:, 512:1024], d1[:, 0:512], d1[:, 512:1024],
                    bts[0][:], bts[1][:], bts[2][:], bts[3][:]]

        # ---------------- V projection ----------------
        # out V[tok, feat]: x stationary (hi/lo big tiles), W streamed.
        xvh = [wres.tile([P, 4, SQ], FP8, name=f"xvh{g}", tag=f"big{g}") for g in range(4)]
        xvl = [wres.tile([P, 4, SQ], FP8, name=f"xvl{g}", tag=f"big{4+g}") for g in range(4)]

        def load_big(tiles, par, g, q=nc.sync, halves=False):
            if halves:
                q.dma_start(tiles[g][:, 0:2, :], prm[par][:, 4 * g : 4 * g + 2, :])
                q.dma_start(tiles[g][:, 2:4, :], prm[par][:, 4 * g + 2 : 4 * g + 4, :])
            else:
                q.dma_start(tiles[g][:], prm[par][:, 4 * g : 4 * g + 4, :])

        # startup: smallest pieces first so the first matmuls begin ASAP
        wvs = {0: stream_tile("wvs0")}
        nc.scalar.dma_start(wvs[0][:, 0:2, :], prm["wvs"][0, :, 0:2, :])
        nc.sync.dma_start(xvh[0][:, 0:2, :], prm["xvh"][:, 0:2, :])
        nc.sync.dma_start(xvl[0][:, 0:2, :], prm["xvl"][:, 0:2, :])
        nc.scalar.dma_start(wvs[0][:, 2:8, :], prm["wvs"][0, :, 2:8, :])
        nc.sync.dma_start(xvh[0][:, 2:4, :], prm["xvh"][:, 2:4, :])
        nc.sync.dma_start(xvl[0][:, 2:4, :], prm["xvl"][:, 2:4, :])
        nc.scalar.dma_start(wvs[0][:, 8:16, :], prm["wvs"][0, :, 8:16, :])
        for g in range(1, 4):
            nc.sync.dma_start(xvh[g][:], prm["xvh"][:, 4 * g : 4 * g + 4, :])
            nc.sync.dma_start(xvl[g][:], prm["xvl"][:, 4 * g : 4 * g + 4, :])
        for fc in range(1, 4):
            wvs[fc] = stream_tile(f"wvs{fc}")
            nc.scalar.dma_start(wvs[fc][:], prm["wvs"][fc])

        gate = res.tile([P, 1], BF16, name="gate", tag="gate")
        for it in range(2):
            ps = proj_ps(f"vps{it}_")
            if it == 1:
                # delay the KV-cache SWDGE loads until V is underway so they
                # don't contend with the V/K stream DMAs at startup
                nc.gpsimd.tensor_copy(gate[:], vv[8][:, 0:1])
                for h in range(HLOC):
                    nc.gpsimd.dma_start(kt[h][:, 0:SC], prm["ckt"][P * h : P * (h + 1), :])
                for t in range(8):
                    nc.gpsimd.dma_start(vv[t][:], prm["cv"][P * t : P * (t + 1), :])
            for qq in range(2):
                wt = wvs[2 * it + qq]
                for u in range(8):
                    for term in range(3):
                        t0 = (2 * u) % 4
                        lt = (xvh, xvl, xvh)[term][u // 2]
                        rhs = wt[:, 2 * u : 2 * u + 2,
                                 (0, 0, 256)[term] : (256, 256, 512)[term]]
                        last = u == 7 and term == 2
                        for m in range(8):
                            nc.tensor.matmul(
                                ps[m][:, 256 * qq : 256 * (qq + 1)],
                                lt[:, t0 : t0 + 2, P * m : P * (m + 1)], rhs,
                                start=(qq == 0 and u == 0 and term == 0),
                                stop=last, perf_mode=DR, skip_group_check=True,
                            )
                            if last and qq == 1:
                                dsl = vv[8 + m][:, 512 * it : 512 * (it + 1)]
                                if m % 2 == 0:
                                    nc.scalar.mul(dsl, ps[m], 1.0 / SW)
                                else:
                                    nc.vector.tensor_scalar_mul(dsl, ps[m], 1.0 / SW)

        # ---------------- K projection, then Q chunk 0 ----------------
        # W resident (hi/lo big tiles), x streamed; out [hd-of-head, tok].
        wkh = [wres.tile([P, 4, DH], FP8, name=f"wkh{g}", tag=f"big{g}") for g in range(4)]
        wkl = [wres.tile([P, 4, DH], FP8, name=f"wkl{g}", tag=f"big{4+g}") for g in range(4)]
        for g in range(4):
            load_big(wkh, "wkh", g)
            load_big(wkl, "wkl", g)
        wqh = [wres.tile([P, 4, DH], FP8, name=f"wqh{g}", tag=f"big{g}") for g in range(4)]
        wql = [wres.tile([P, 4, DH], FP8, name=f"wql{g}", tag=f"big{4+g}") for g in range(4)]

        def proj_wx(wh, wl, x_par, its, dest_fn, scale, bias2, pref):
            for it in its:
                ps = proj_ps(f"{pref}ps{it}_")
                for qq in range(2):
                    xst = stream_tile(f"{pref}x{it}_{qq}")
                    load_stream(xst, x_par, 2 * it + qq)
                    for u in range(8):
                        for term in range(3):
                            t0 = (2 * u) % 4
                            lt = (wh, wl, wh)[term][u // 2]
                            rhs = xst[:, 2 * u : 2 * u + 2,
                                      (0, 0, 256)[term] : (256, 256, 512)[term]]
                            last = u == 7 and term == 2
                            for m in range(8):
                                nc.tensor.matmul(
                                    ps[m][:, 256 * qq : 256 * (qq + 1)],
                                    lt[:, t0 : t0 + 2, P * m : P * (m + 1)], rhs,
                                    start=(qq == 0 and u == 0 and term == 0),
                                    stop=last, perf_mode=DR, skip_group_check=True,
                                )
                                if last and qq == 1:
                                    dsl = dest_fn(m, it)
                                    if m % 2 == 0:
                                        nc.scalar.activation(
                                            dsl, ps[m], IDENT,
                                            bias=bias2[:, m : m + 1], scale=scale,
                                        )
                                    else:
                                        nc.vector.tensor_scalar(
                                            dsl, ps[m], scale, bias2[:, m : m + 1],
                                            op0=MUL, op1=ADD,
                                        )

        proj_wx(
            wkh, wkl, "xks", range(2),
            lambda m, it: kt[m][:, SC + 512 * it : SC + 512 * (it + 1)],
            1.0 / SW, bk2, "k",
        )
        for g in range(4):
            load_big(wqh, "wqh", g)
            load_big(wql, "wql", g)
        proj_wx(
            wqh, wql, "xqs", range(1),
            lambda m, it: qt[m][:, 0:512],
            1.0 / SWQ, bq2, "q",
        )

        # ---------------- attention ----------------
        def q_deferred(mg2):
            # deferred Q projection (chunk 1, token cols 512:1024) for the
            # head pair (2*mg2, 2*mg2+1); PSUM banks x0/x1 are free until
            # the out-projection. Head a accumulates its two 256-col token
            # chunks in the two halves of one bank (shared zero region).
            p0 = ps8.tile([P, 512], F32, name=f"qdp0_{mg2}", tag="x0")
            p1 = ps8.tile([P, 512], F32, name=f"qdp1_{mg2}", tag="x1")
            for qq in range(2):
                xst = stream_tile(f"qd{mg2}_{qq}")
                load_stream(xst, "xqs", 2 + qq, q=nc.sync)
                for u in range(8):
                    for term in range(3):
                        t0 = (2 * u) % 4
                        lt = (wqh, wql, wqh)[term][u // 2]
                        rhs = xst[:, 2 * u : 2 * u + 2,
                                  (0, 0, 256)[term] : (256, 256, 512)[term]]
                        last = u == 7 and term == 2
                        for i in range(2):
                            m = 2 * mg2 + i
                            ps = (p0, p1)[i]
                            nc.tensor.matmul(
                                ps[:, 256 * qq : 256 * (qq + 1)],
                                lt[:, t0 : t0 + 2, P * m : P * (m + 1)], rhs,
                                start=(qq == 0 and u == 0 and term == 0),
                                stop=last, perf_mode=DR, skip_group_check=True,
                            )
                            if last and qq == 1:
                                dsl = qt[m][:, 512:1024]
                                if i % 2 == 0:
                                    nc.scalar.activation(
                                        dsl, ps, IDENT,
                                        bias=bq2[:, m : m + 1], scale=1.0 / SWQ,
                                    )
                                else:
                                    nc.vector.tensor_scalar(
                                        dsl, ps, 1.0 / SWQ, bq2[:, m : m + 1],
                                        op0=MUL, op1=ADD,
                                    )

        # Wo^T hi/lo, head-pair-major: [P, 8, D] with dim1 = 2j+s
        wo8h = [wres.tile([P, 2, D], FP8, name=f"wo8h{j}", tag=f"big{j}") for j in range(4)]
        wo8l = [wres.tile([P, 2, D], FP8, name=f"wo8l{j}", tag=f"big{4+j}") for j in range(4)]

        def wo_prefetch():
            for j in range(4):
                nc.sync.dma_start(wo8h[j][:], prm["wo8h"][:, 2 * j : 2 * j + 2, :])
                nc.sync.dma_start(wo8l[j][:], prm["wo8l"][:, 2 * j : 2 * j + 2, :])

        pending = []  # deferred (o_ps, d_all, h, c) normalizes
        pvq = []  # cross-chunk PV pipeline (emitted PDP pairs behind scores)
        PDP = 1

        def flush_normalize():
            o_ps, d_all, h, c = pending.pop(0)
            rec = dnp.tile([P, 512], F32, name=f"rec{h}_{c}", tag="rec")
            nc.vector.reciprocal(rec[:], d_all[:])
            atb = abf.tile([P, 512], BF16, name=f"atb{h}_{c}", tag="atb")
            nc.vector.scalar_tensor_tensor(atb[:], o_ps[:], SA, rec[:], op0=MUL, op1=MUL)
            j, s = h // 2, h % 2
            hsl = ath[j][:, s, 512 * c : 512 * (c + 1)]
            nc.gpsimd.tensor_copy(hsl, atb[:])
            nc.gpsimd.tensor_sub(atl[j][:, s, 512 * c : 512 * (c + 1)], atb[:], hsl)

        def attn(h, c, idx):
            n_full = 8 + 4 * c
            n_kv = n_full + 4
            q_sl = slice(512 * c, 512 * (c + 1))
            ptb = ptp.tile([P, 16, 512], BF16, name=f"ptb{h}_{c}", tag="ptb")
            ts = trp.tile([P, 6, 512], BF16, name=f"ts{h}_{c}", tag="ts")
            acc = accp.tile([P, 512], BF16, name=f"acc{h}_{c}", tag="acc")
            d_all = dnp.tile([P, 512], F32, name=f"d{h}_{c}", tag="d")
            o_ps = ps8.tile([P, 512], F32, name=f"o{h}_{c}", tag=f"o{idx % 2}")

            def tree_root():
                if n_full == 8:
                    nc.vector.tensor_add(ts[:, 4, :], ts[:, 0, :], ts[:, 1, :])
                    nc.vector.tensor_add(ts[:, 5, :], ts[:, 2, :], ts[:, 3, :])
                    nc.vector.tensor_add(acc[:], ts[:, 4, :], ts[:, 5, :])
                else:  # 12 full tiles -> 6 pairs
                    nc.vector.tensor_add(ts[:, 0, :], ts[:, 0, :], ts[:, 1, :])
                    nc.vector.tensor_add(ts[:, 2, :], ts[:, 2, :], ts[:, 3, :])
                    nc.vector.tensor_add(ts[:, 4, :], ts[:, 4, :], ts[:, 5, :])
                    nc.vector.tensor_add(ts[:, 0, :], ts[:, 0, :], ts[:, 2, :])
                    nc.vector.tensor_add(acc[:], ts[:, 0, :], ts[:, 4, :])

            def scores_pair(u):
                # two score tiles per double-bank PSUM tile; full pairs get
                # one [P,1024] exp straight into adjacent ptb slots
                dp = ps8.tile([P, 1024], F32, name=f"st{h}_{c}_{u}", tag=f"d{u % 2}")
                for half in range(2):
                    g = 2 * u + half
                    j = g - n_full
                    base = 512 * half
                    if j < 0:
                        nc.tensor.matmul(
                            dp[:, base : base + 512],
                            kt[h][:, P * g : P * (g + 1)], qt[h][:, q_sl],
                            start=True, stop=True,
                        )
                    else:
                        o0 = P * j
                        nc.tensor.matmul(
                            dp[:, base + o0 : base + 512],
                            kt[h][:, P * g : P * (g + 1)],
                            qt[h][:, 512 * c + o0 : 512 * (c + 1)],
                            start=True, stop=True,
                        )
                g = 2 * u + 1
                if g < n_full:
                    nc.scalar.activation(ptb[:, 2 * u : 2 * u + 2, :], dp[:], EXP)
                    nc.vector.tensor_add(
                        ts[:, u, :], ptb[:, g - 1, :], ptb[:, g, :]
                    )
                    if g == n_full - 1:
                        tree_root()
                else:
                    first_diag = 2 * u == n_full
                    if first_diag:
                        # j=0 spans the full half; one exp covers both tiles
                        # (the 128 garbage cols between are never read)
                        nc.scalar.activation(ptb[:, 2 * u : 2 * u + 2, :], dp[:], EXP)
                    for half in range(2):
                        g = 2 * u + half
                        o0 = P * (g - n_full)
                        base = 512 * half
                        if not first_diag:
                            nc.scalar.activation(
                                ptb[:, g, o0:512], dp[:, base + o0 : base + 512], EXP
                            )
                        nc.vector.tensor_mul(
                            ptb[:, g, o0 : o0 + P], ptb[:, g, o0 : o0 + P], tri[:]
                        )
                        nc.vector.tensor_add(
                            acc[:, o0:512], acc[:, o0:512], ptb[:, g, o0:512]
                        )

            def pv(g):
                j = g - n_full
                if j < 0:
                    nc.tensor.matmul(
                        o_ps[:], vv[g][:, P * h : P * (h + 1)], ptb[:, g, :],
                        start=(g == 0), stop=False,
                    )
                else:
                    o0 = P * j
                    # retiring query block: last writer of cols [o0, o0+128)
                    nc.tensor.matmul(
                        o_ps[:, o0 : o0 + P], vv[g][:, P * h : P * (h + 1)],
                        ptb[:, g, o0 : o0 + P], start=False, stop=True,
                    )
                    if o0 + P < 512:
                        nc.tensor.matmul(
                            o_ps[:, o0 + P : 512], vv[g][:, P * h : P * (h + 1)],
                            ptb[:, g, o0 + P : 512], start=False, stop=(j == 3),
                        )

            def finish():
                nc.gpsimd.partition_all_reduce(
                    d_all[:], acc[:], channels=P, reduce_op=bass_isa.ReduceOp.add
                )
                pending.append((o_ps, d_all, h, c))
                if len(pending) >= 2:
                    flush_normalize()

            n_pairs = n_kv // 2
            for u in range(n_pairs):
                scores_pair(u)

                def mk(g0=2 * u, fin=(finish if u == n_pairs - 1 else None)):
                    def emit():
                        pv(g0)
                        pv(g0 + 1)
                        if fin is not None:
                            fin()
                    return emit

                pvq.append(mk())
                while len(pvq) > PDP:
                    pvq.pop(0)()

        def outproj(c):
            for m in range(16):
                op2 = ps8.tile([P, 512], F32, name=f"op{m}_{c}", tag=f"x{m % 2}")
                ob = outs.tile([P, 512], BF16, name=f"ob{m}_{c}", tag="ob")
                for t2 in range(2):
                    opsl = op2[:, 256 * t2 : 256 * (t2 + 1)]
                    cols = slice(512 * c + 256 * t2, 512 * c + 256 * (t2 + 1))
                    for j in range(4):
                        for term in range(3):
                            lt = (wo8h, wo8h, wo8l)[term][j]
                            rt = (ath, atl, ath)[term][j]
                            nc.tensor.matmul(
                                opsl, lt[:, :, P * m : P * (m + 1)], rt[:, :, cols],
                                start=(t2 == 0 and j == 0 and term == 0),
                                stop=(j == 3 and term == 2),
                                perf_mode=DR, skip_group_check=True,
                            )
                if m % 2 == 0:
                    nc.scalar.mul(ob[:], op2[:], 1.0 / (SA * SW))
                else:
                    nc.vector.tensor_scalar_mul(ob[:], op2[:], 1.0 / (SA * SW))
                nc.sync.dma_start(
                    prm["outT"][P * m : P * (m + 1), 512 * c : 512 * (c + 1)], ob[:]
                )

        idx = 0
        for h in range(HLOC):
            attn(h, 0, idx)
            idx += 1
            if h >= 5:
                q_deferred(h - 5)
            if h == 7:
                wo_prefetch()
        for h in range(HLOC):
            attn(h, 1, idx)
            idx += 1
            if h == 0:
                q_deferred(3)
        while pvq:
            pvq.pop(0)()
        while pending:
            flush_normalize()
        outproj(0)
        outproj(1)


def build():
    nc = bacc.Bacc(None, target_bir_lowering=False)
    prm = {}
    for n, shape, dt in (
        ("xvh", [P, 16, SQ], FP8), ("xvl", [P, 16, SQ], FP8),
        ("wkh", [P, 16, DH], FP8), ("wkl", [P, 16, DH], FP8),
        ("wqh", [P, 16, DH], FP8), ("wql", [P, 16, DH], FP8),
        ("wvs", [4, P, 16, 512], FP8),
        ("xks", [4, P, 16, 512], FP8),
        ("xqs", [4, P, 16, 512], FP8),
        ("wo8h", [P, 8, D], FP8), ("wo8l", [P, 8, D], FP8),
        ("ckt", [DH, SC], BF16),
        ("cv", [SC, DH], BF16),
        ("tri", [P, P], BF16),
        ("bq2", [P, 8], F32),
        ("bk2", [P, 8], F32),
    ):
        prm[n] = nc.declare_dram_parameter(n, shape, dt, isOutput=False)
    prm["outT"] = nc.declare_dram_parameter("outT", [D, SQ], BF16, isOutput=True)
    with tile.TileContext(nc) as tc:
        _emit(tc, nc, prm)
    nc.compile()
    return nc


def _hl(a):
    """Split a float32 array into e4m3 hi + lo parts (hi + lo ~ a)."""
    a32 = np.ascontiguousarray(np.asarray(a, np.float32))
    h = np.clip(a32, -240, 240).astype(E4)
    l = (a32 - h.astype(np.float32)).astype(E4)
    return h, l


def make_in_maps(query, key, value, cached_k, cached_v, Wq, bq, Wk, bk, Wv, bv, Wo, bo):
    """Per-core host prep: slice + transpose + hi/lo fp8 splits."""
    s = float(np.sqrt(HD))
    tri = np.triu(np.ones((P, P), dtype=np.float32)).astype(BF)

    def t16(a):  # [D, N] -> [P, 16, N] contraction-tiled
        return np.ascontiguousarray(a.reshape(16, P, a.shape[1]).transpose(1, 0, 2))

    def pack_stream(h, l):  # two [P, 16, N] -> [4, P, 16, 512] hi|lo per 256-chunk
        n4 = h.shape[2] // 4
        out = np.empty((4, P, 16, 512), h.dtype)
        for c4 in range(4):
            out[c4, :, :, 0:256] = h[:, :, n4 * c4 : n4 * c4 + 256]
            out[c4, :, :, 256:512] = l[:, :, n4 * c4 : n4 * c4 + 256]
        return out

    in_maps = []
    for c in range(NCORES):
        b, h2 = c // 2, c % 2
        hs = slice(DH * h2, DH * (h2 + 1))
        m = {}
        xvh, xvl = _hl(value[b].T)
        m["xvh"], m["xvl"] = t16(xvh), t16(xvl)
        wkh, wkl = _hl(Wk[hs].T * SW)
        m["wkh"], m["wkl"] = t16(wkh), t16(wkl)
        wqh, wql = _hl(Wq[hs].T / s * SWQ)
        m["wqh"], m["wql"] = t16(wqh), t16(wql)
        wvh, wvl = _hl(Wv[hs].T * SW)
        m["wvs"] = pack_stream(t16(wvh), t16(wvl))
        xkh, xkl = _hl(key[b].T)
        m["xks"] = pack_stream(t16(xkh), t16(xkl))
        xqh, xql = _hl(query[b].T)
        m["xqs"] = pack_stream(t16(xqh), t16(xql))
        woh, wol = _hl(Wo[:, hs].T * SW)
        m["wo8h"] = np.ascontiguousarray(woh.reshape(8, P, D).transpose(1, 0, 2))
        m["wo8l"] = np.ascontiguousarray(wol.reshape(8, P, D).transpose(1, 0, 2))
        m["ckt"] = np.ascontiguousarray(cached_k[b][:, hs].T).astype(BF)
        m["cv"] = np.ascontiguousarray(cached_v[b][:, hs] - bv[hs]).astype(BF)
        m["tri"] = tri
        m["bq2"] = np.ascontiguousarray((bq[hs] / s).reshape(8, P).T.astype(np.float32))
        m["bk2"] = np.ascontiguousarray(bk[hs].reshape(8, P).T.astype(np.float32))
        in_maps.append(m)
    return in_maps


_NC_CACHE = []


def get_nc():
    if not _NC_CACHE:
        _NC_CACHE.append(build())
    return _NC_CACHE[0]


def assemble(results, bo):
    out = np.empty((4, SQ, D), dtype=np.float32)
    for b in range(4):
        acc = results[2 * b]["outT"].astype(np.float32) + results[
            2 * b + 1
        ]["outT"].astype(np.float32)  # [D, SQ]
        out[b] = acc.T + bo[None, :]
    return out


def kernel(query, key, value, cached_k, cached_v, Wq, bq, Wk, bk, Wv, bv, Wo, bo):
    query = np.asarray(query, dtype=np.float32)
    key = np.asarray(key, dtype=np.float32)
    value = np.asarray(value, dtype=np.float32)
    cached_k = np.asarray(cached_k, dtype=np.float32)
    cached_v = np.asarray(cached_v, dtype=np.float32)
    Wq, bq = np.asarray(Wq, np.float32), np.asarray(bq, np.float32)
    Wk, bk = np.asarray(Wk, np.float32), np.asarray(bk, np.float32)
    Wv, bv = np.asarray(Wv, np.float32), np.asarray(bv, np.float32)
    Wo, bo = np.asarray(Wo, np.float32), np.asarray(bo, np.float32)

    nc = get_nc()
    in_maps = make_in_maps(
        query, key, value, cached_k, cached_v, Wq, bq, Wk, bk, Wv, bv, Wo, bo
    )
    res = run_bass_kernel_spmd(nc, in_maps, list(range(NCORES)))
    return assemble(res.results, bo + bv @ Wo.T)
